# revision 7
# baseline (speedup 1.0000x reference)
"""Multi-head attention kernel for Trainium2, data-parallel over batch on 8 NeuronCores.

Reference computation (per batch element b of 8):
    qkv = x @ W_qkv.T + b_qkv            [1024, 2304]
    q, k, v = split(qkv)                 each [1024, 768], 12 heads x 64
    S_h = q_h @ k_h.T * d**-0.5          [1024, 1024] per head
    A_h = softmax(S_h, axis=-1)
    o_h = A_h @ v_h                      [1024, 64]
    y = concat(o) @ W_out.T + b_out      [1024, 768]

v2 strategy (one batch element per core, zero communication):
  * q/k projection in fp8 DoubleRow with an x-RESIDUAL second pass
    (x ~ x8 + fp8(x - x8)), which cancels the x-side fp8 quantization
    noise: measured rel-err 1.62e-2 in simulation vs 1.54e-2 baseline.
  * Scores S^T in fp8 DoubleRow: q/k are re-quantized to fp8 and packed
    [32 partitions x 2 k-tiles]; a DoubleRow matmul emits 512 cols in
    256 cycles (0.5 cyc/col measured), halving the S cost vs bf16.
  * A@V and both v/out projections stay bf16 (fp8 there would put ~3.6%
    noise linearly on the output).
  * V carries a leading ones column so A@V yields softmax denominators
    in PSUM partition 0; reciprocal_approx_fast reads that PSUM row
    directly (base 0 works; the known misread is at base 64).
  * Normalization is pair-batched: o^T is cast to bf16 raw, denominators
    go through recip -> DRAM -> partition-broadcast DMA, one
    tensor_tensor multiply per head PAIR produces otn[fc] (the out-proj
    stationary), saving ~30us of DVE vs per-head normalize.
  * Out-projection is staged to keep the PE fed mid-attention:
    fc0+1 after head 3, fc2+3 after head 7, fc4 after head 9 (all as
    PSUM->SBUF f32 partials), only fc5 + final add + store in the tail.
  * The PE p-state drops 2.4->1.2GHz after any idle gap and takes 3us
    of continuous work to recover, so a junk-work generator (DoubleRow
    matmuls into a dead PSUM half-bank) backstops the filler queue.
"""

import numpy as np
import ml_dtypes

B, N, D, H, HD = 8, 1024, 768, 12, 64
NCORES = 8
SCALE = float(D) ** -0.5
DC = D // 128            # 6 chunks of 128 for d=768
IC = N // 128            # 8 token chunks
KC = N // 128            # 8 key chunks
NI = 3                   # 256-deep contraction double-chunks for fp8 DR


def _build(has_bqkv: bool):
    import concourse.bass as bass
    import concourse.mybir as mybir
    import concourse.tile as tile
    from concourse import bacc

    f32 = mybir.dt.float32
    bf16 = mybir.dt.bfloat16
    fp8 = mybir.dt.float8e4
    Exp = mybir.ActivationFunctionType.Exp
    DR = mybir.MatmulPerfMode.DoubleRow
    Add = mybir.AluOpType.add
    Mult = mybir.AluOpType.mult

    nc = bacc.Bacc("TRN2", target_bir_lowering=False, debug=False,
                   num_devices=NCORES)

    xa8_ext = [nc.dram_tensor(f"xa8_{i}", [128, 2 * N], fp8, kind="ExternalInput")
               for i in range(NI)]
    xr8_ext = [nc.dram_tensor(f"xr8_{i}", [128, 2 * N], fp8, kind="ExternalInput")
               for i in range(NI)]
    wqf_ext = [nc.dram_tensor(f"wqf{i}", [128, 4 * D], fp8, kind="ExternalInput")
               for i in range(NI)]
    xT_ext = nc.dram_tensor("xT", [D, N], bf16, kind="ExternalInput")
    wvT_ext = nc.dram_tensor("wvT", [D, D], bf16, kind="ExternalInput")
    woutT_ext = nc.dram_tensor("woutT", [D, D], bf16, kind="ExternalInput")
    if has_bqkv:
        bqkv_ext = nc.dram_tensor("bqkv", [2 * D], f32, kind="ExternalInput")
        bv16_ext = nc.dram_tensor("bv16", [D], bf16, kind="ExternalInput")
    out_ext = nc.dram_tensor("out", [N, D], f32, kind="ExternalOutput")
    rdram = nc.dram_tensor("recip_scratch", [H, N], f32)

    with tile.TileContext(nc) as tc:
        with (
            tc.tile_pool(name="w", bufs=1) as wpool,
            tc.tile_pool(name="act", bufs=1) as apool,
            tc.tile_pool(name="es", bufs=7) as espool,
            tc.tile_pool(name="rows", bufs=2) as rowpool,
            tc.tile_pool(name="bc", bufs=2) as bcpool,
            tc.tile_pool(name="y", bufs=3) as ypool,
            tc.tile_pool(name="ps", bufs=1, space="PSUM") as pspool,
        ):
            # ---- input DMAs: qk-proj operands first for earliest PE start ----
            xa8 = [wpool.tile([128, 2 * N], fp8, tag=f"xa8_{i}", name=f"xa8_{i}") for i in range(NI)]
            xr8 = [wpool.tile([128, 2 * N], fp8, tag=f"xr8_{i}", name=f"xr8_{i}") for i in range(NI)]
            wqf = [wpool.tile([128, 4 * D], fp8, tag=f"wqf{i}", name=f"wqf{i}") for i in range(NI)]
            xT = [wpool.tile([128, N], bf16, tag=f"xT{i}", name=f"xT{i}") for i in range(DC)]
            wv = [wpool.tile([128, D], bf16, tag=f"wv{i}", name=f"wv{i}") for i in range(DC)]
            wo = [wpool.tile([128, D], bf16, tag=f"wo{i}", name=f"wo{i}") for i in range(DC)]
            for i in range(NI):
                nc.sync.dma_start(out=xa8[i][:], in_=xa8_ext[i][:, :])
                nc.gpsimd.dma_start(out=wqf[i][:], in_=wqf_ext[i][:, :])
            for i in range(NI):
                nc.sync.dma_start(out=xr8[i][:], in_=xr8_ext[i][:, :])
            for dc in range(DC):
                nc.sync.dma_start(out=xT[dc][:], in_=xT_ext[dc * 128:(dc + 1) * 128, :])
                nc.gpsimd.dma_start(out=wv[dc][:], in_=wvT_ext[dc * 128:(dc + 1) * 128, :])
            for dc in range(DC):
                nc.gpsimd.dma_start(out=wo[dc][:], in_=woutT_ext[dc * 128:(dc + 1) * 128, :])

            if has_bqkv:
                bqk_t = wpool.tile([128, 2 * DC], f32, tag="bqk")
                for jc in range(2 * DC):
                    nc.sync.dma_start(
                        out=bqk_t[:, jc:jc + 1],
                        in_=bqkv_ext[jc * 128:(jc + 1) * 128][:, None])
                bv_t = wpool.tile([1, D], bf16, tag="bv")
                nc.sync.dma_start(out=bv_t[:], in_=bv16_ext[:][None, :])
                ones_t = wpool.tile([1, 128], bf16, tag="ones")
                nc.vector.memset(ones_t[:], 1.0)

            xa3 = [t.rearrange("p (two n) -> p two n", two=2) for t in xa8]
            xr3 = [t.rearrange("p (two n) -> p two n", two=2) for t in xr8]
            wqf3 = [t.rearrange("p (two n) -> p two n", two=2) for t in wqf]

            # ---- q/k fp8 DR-packed tiles: chunk jc (0..5 q, 6..11 k) ----
            # feature 64*hh + 32*t + p  ->  partition 64*hh + p, k-tile t
            qk8 = [apool.tile([128, 2, N], fp8, tag=f"qk8_{j}", name=f"qk8_{j}")
                   for j in range(2 * DC)]
            f8s = [apool.tile([128, N], fp8, tag=f"f8s{i}", name=f"f8s{i}")
                   for i in range(2)]

            # PSUM budget (8 banks): A,B = sps double-buffer (2+2), C = ot (2),
            # D = two [128,512] one-bank tiles for fillers + junk.
            def big_ps(tag, name):
                return pspool.tile([128, N], f32, tag=tag, name=name)

            def half_ps(name):
                return pspool.tile([128, 512], f32, tag="D", bufs=2, name=name)

            def qk_mms(ps, jc, s0, cols):
                """fp8 DR passes (main + x-residual) for features chunk jc.

                ps is a [128, cols] PSUM region; matmuls are emitted in
                512-wide pieces (PSUM bank limit)."""
                for i in range(NI):
                    for x3 in (xa3, xr3):
                        for ih in range(cols // 512):
                            yield nc.tensor.matmul(
                                ps[:, ih * 512:(ih + 1) * 512],
                                wqf3[i][:, :, jc * 128:(jc + 1) * 128],
                                x3[i][:, :, s0 + ih * 512:s0 + (ih + 1) * 512],
                                start=(i == 0 and x3 is xa3),
                                stop=(i == NI - 1 and x3 is xr3),
                                perf_mode=DR)

            def qk_pack(jc, ps, s, e, eng):
                """Cast PSUM f32 -> fp8 and shuffle into DR layout."""
                fs = f8s[jc % 2]
                if has_bqkv:
                    nc.vector.tensor_scalar_add(fs[:, s:e], ps[:, 0:e - s],
                                                bqk_t[:, jc:jc + 1])
                else:
                    nc.vector.tensor_copy(fs[:, s:e], ps[:, 0:e - s])
                t = qk8[jc]
                eng.dma_start(out=t[0:32, 0, s:e], in_=fs[0:32, s:e])
                eng.dma_start(out=t[0:32, 1, s:e], in_=fs[32:64, s:e])
                eng.dma_start(out=t[64:96, 0, s:e], in_=fs[64:96, s:e])
                eng.dma_start(out=t[64:96, 1, s:e], in_=fs[96:128, s:e])

            def qk_chunk_full(jc, tag, eng):
                ps = big_ps(tag, f"qkps{jc}")
                for _ in qk_mms(ps, jc, 0, N):
                    pass
                qk_pack(jc, ps, 0, N, eng)

            def gen_qk_chunk(jc):
                """Filler generator: one chunk as 2 half-bank units."""
                for ih in range(2):
                    ps = half_ps(f"qkh{jc}_{ih}")
                    yield from qk_mms(ps, jc, ih * 512, 512)
                    qk_pack(jc, ps, ih * 512, (ih + 1) * 512,
                            nc.sync if ih == 0 else nc.gpsimd)

            # ---- v : [1024 tokens, 12 heads x (64+1)] with trailing ones col ----
            v = [apool.tile([128, H, HD + 1], bf16, tag=f"v{i}", name=f"v{i}") for i in range(IC)]
            for ic in range(IC):
                nc.vector.memset(v[ic][:, :, HD:HD + 1], 1.0)
            vsplits = [(0, 512), (512, 768)]

            def v_mms(ps, ic, s, e):
                if has_bqkv:
                    yield nc.tensor.matmul(ps[:, 0:e - s], ones_t[:],
                                           bv_t[:, s:e], start=True, stop=False)
                for dc in range(DC):
                    yield nc.tensor.matmul(
                        ps[:, 0:e - s],
                        xT[dc][:, ic * 128:(ic + 1) * 128],
                        wv[dc][:, s:e],
                        start=(dc == 0 and not has_bqkv), stop=(dc == DC - 1))

            def v_store(ic, ps, s, e):
                h0, h1 = s // HD, e // HD
                nc.vector.tensor_copy(
                    v[ic][:, h0:h1, 0:HD],
                    ps[:, 0:e - s].rearrange("p (h e) -> p h e", h=h1 - h0))

            def v_chunk(ic, tag):
                ps = big_ps(tag, f"vps{ic}")
                for s, e in vsplits:
                    for _ in v_mms(ps[:, s:], ic, s, e):
                        pass
                for s, e in vsplits:
                    v_store(ic, ps[:, s:], s, e)

            def gen_v_chunk(ic):
                for s, e in vsplits:
                    ps = half_ps(f"vh{ic}_{s}")
                    yield from v_mms(ps, ic, s, e)
                    v_store(ic, ps, s, e)

            # ---- attention: flat software pipeline over (head, kc) ----
            otu16 = {}

            fillers = []

            def fill(n):
                while n > 0 and fillers:
                    try:
                        next(fillers[0])
                        n -= 1
                    except StopIteration:
                        fillers.pop(0)

            def junk_gen():
                while True:
                    for jc in range(2 * DC):
                        ps = half_ps(f"junk{jc}")
                        for ih in range(2):
                            yield nc.tensor.matmul(
                                ps[:],
                                wqf3[0][:, :, jc * 128:(jc + 1) * 128],
                                xa3[0][:, :, ih * 512:(ih + 1) * 512],
                                start=True, stop=True, perf_mode=DR)

            def smm(h, kc, sps):
                qt, kt = qk8[h // 2], qk8[DC + h // 2]
                p0 = 64 * (h % 2)
                for ih in range(2):
                    nc.tensor.matmul(
                        sps[:, ih * 512:(ih + 1) * 512],
                        kt[p0:p0 + 32, :, kc * 128:(kc + 1) * 128],
                        qt[p0:p0 + 32, :, ih * 512:(ih + 1) * 512],
                        start=True, stop=True, perf_mode=DR)

            def avmm(h, kc, et, ot):
                for ih in range(2):
                    nc.tensor.matmul(
                        ot[0:HD + 1, ih * 512:(ih + 1) * 512],
                        v[kc][:, h, :],
                        et[:, ih * 512:(ih + 1) * 512],
                        start=(kc == 0), stop=(kc == KC - 1))

            def head_epilogue(h, ot):
                # denominator staging first (it gates normalize); the custom
                # recip op misreads PSUM rows at partition base 64, so stage
                # to a base-0 SBUF row first.
                t = h // 2
                drow = rowpool.tile([1, N], f32, tag="drow", bufs=2, name=f"drow{h}")
                nc.vector.tensor_copy(drow[:], ot[HD:HD + 1, :])
                rc32 = rowpool.tile([1, N], f32, tag="rc32", bufs=2, name=f"rc32_{h}")
                nc.vector.reciprocal_approx_fast(rc32[:], drow[:])
                eng = nc.sync if h % 2 == 0 else nc.gpsimd
                eng.dma_start(out=rdram[h:h + 1, :], in_=rc32[:])
                if h % 2 == 0:
                    otu16[t] = rowpool.tile([128, N], bf16, tag="otu16", bufs=2,
                                            name=f"otu16_{t}")
                with nc.allow_low_precision(reason="bf16 raw o; 2e-2 gate"):
                    nc.vector.tensor_copy(
                        otu16[t][64 * (h % 2):64 * (h % 2) + 64, :],
                        ot[0:HD, :])
                bch = bcpool.tile([128, N], f32, tag="bc", bufs=2, name=f"bc{t}") \
                    if h % 2 == 0 else bcs[t]
                if h % 2 == 0:
                    bcs[t] = bch
                eng.dma_start(
                    out=bch[64 * (h % 2):64 * (h % 2) + 64, :],
                    in_=rdram[h:h + 1, :].to_broadcast((64, N)))

            bcs = {}

            def normalize_pair(t):
                with nc.allow_low_precision(reason="bf16 normalized o; 2e-2 gate"):
                    nc.vector.tensor_tensor(
                        out=otn[t][:], in0=otu16.pop(t)[:], in1=bcs.pop(t)[:],
                        op=Mult)

            # ---- output projection ----
            # ypart[ic] accumulates fc stages in f32 SBUF via filler units;
            # tail adds fc5 from PSUM.
            otn = [apool.tile([128, N], bf16, tag=f"otn{t}", name=f"otn{t}") for t in range(DC)]
            ypart = [apool.tile([128, D], f32, tag=f"yp{ic}", name=f"yp{ic}") for ic in range(IC)]

            def gen_outproj_stage(ic, fcs, first_stage):
                for s, e in vsplits:
                    ps = half_ps(f"yh{ic}_{s}_{fcs[0]}")
                    for j, fc in enumerate(fcs):
                        yield nc.tensor.matmul(
                            ps[:, 0:e - s],
                            otn[fc][:, ic * 128:(ic + 1) * 128],
                            wo[fc][:, s:e],
                            start=(j == 0), stop=(j == len(fcs) - 1))
                    if first_stage:
                        nc.vector.tensor_copy(ypart[ic][:, s:e], ps[:, 0:e - s])
                    else:
                        nc.vector.tensor_tensor(
                            out=ypart[ic][:, s:e], in0=ps[:, 0:e - s],
                            in1=ypart[ic][:, s:e], op=Add)

            def outproj_tail_mm(ic, ps):
                for s, e in vsplits:
                    nc.tensor.matmul(
                        ps[:, s:e],
                        otn[5][:, ic * 128:(ic + 1) * 128],
                        wo[5][:, s:e],
                        start=True, stop=True)

            def outproj_finish(ic, ps):
                ysb = ypool.tile([128, D], f32, tag="y", name=f"y{ic}")
                nc.vector.tensor_tensor(
                    out=ysb[:], in0=ps[:, 0:D], in1=ypart[ic][:], op=Add)
                eng = nc.sync if ic % 2 == 0 else nc.gpsimd
                eng.dma_start(out=out_ext[ic * 128:(ic + 1) * 128, :], in_=ysb[:])

            # ---- phase A: q/k chunks for heads 0,1, then v chunks with
            # head-0 S/exp interleaved so ACT starts early ----
            tags = ["A", "B", "C"]
            ets = {}
            ots = {}

            def emit_s(step):
                h, kc = divmod(step, KC)
                sps = big_ps("A" if step % 2 == 0 else "B", f"sps{h}_{kc}")
                smm(h, kc, sps)
                et = espool.tile([128, N], bf16, tag="es", name=f"es{h}_{kc}")
                nc.scalar.activation(et[:], sps[:], Exp, scale=SCALE)
                ets[step] = et

            qk_chunk_full(0, "A", nc.sync)
            qk_chunk_full(DC, "B", nc.gpsimd)
            for ic in range(6):
                v_chunk(ic, "C")
                if ic >= 1:
                    emit_s(ic - 1)    # S(0, 0..4) between v chunks

            # filler queue in deadline order.
            fillers.append(gen_v_chunk(6))
            fillers.append(gen_v_chunk(7))
            for jc in [1, DC + 1, 2, DC + 2, 3, DC + 3, 4, DC + 4, 5, DC + 5]:
                fillers.append(gen_qk_chunk(jc))
            junk = junk_gen()

            def fill_or_junk(n):
                if fillers:
                    fill(n)
                else:
                    for _ in range(n):
                        next(junk)

            emitted = 5   # S(0,0..4) already issued during phase A
            for step in range(H * KC + 1):
                if step < H * KC and step >= emitted:
                    emit_s(step)
                    emitted = step + 1
                boundary = False
                if step > 0:
                    hp, kcp = divmod(step - 1, KC)
                    if kcp == 0:
                        ots[hp] = big_ps("C", f"ot{hp}")
                    avmm(hp, kcp, ets.pop(step - 1), ots[hp])
                    if kcp == KC - 1:
                        boundary = True
                        head_epilogue(hp, ots.pop(hp))
                        if hp % 2 == 1:
                            normalize_pair(hp // 2)
                        if hp == 3:
                            for ic in range(IC):
                                fillers.append(gen_outproj_stage(ic, [0, 1], True))
                        elif hp == 7:
                            for ic in range(IC):
                                fillers.append(gen_outproj_stage(ic, [2, 3], False))
                        elif hp == 9:
                            for ic in range(IC):
                                fillers.append(gen_outproj_stage(ic, [4], False))
                        # pre-emit the next step's S so ACT keeps cadence
                        if step + 1 < H * KC and step + 1 >= emitted:
                            emit_s(step + 1)
                            emitted = step + 2
                fill_or_junk(10 if boundary else (4 if step < 12 else 3))

            # tail: only fc5 remains, gated on otn[5].
            fill(10 ** 9)  # flush remaining real fillers
            tps = {}
            for ic in range(3):
                tps[ic] = big_ps(tags[ic % 3], f"yt{ic}")
                outproj_tail_mm(ic, tps[ic])
            for ic in range(3, IC):
                outproj_finish(ic - 3, tps.pop(ic - 3))
                tps[ic] = big_ps(tags[ic % 3], f"yt{ic}")
                outproj_tail_mm(ic, tps[ic])
            for ic in (IC - 3, IC - 2, IC - 1):
                outproj_finish(ic, tps.pop(ic))

    nc.compile()
    return nc


def _prepare(x, W_qkv, b_qkv, W_out, b_out):
    """Build the compiled graph and per-core input maps."""
    bfd = ml_dtypes.bfloat16
    f8 = ml_dtypes.float8_e4m3
    xT = np.ascontiguousarray(np.transpose(x, (0, 2, 1)))                # [B, D, N] f32
    wqkvT = np.ascontiguousarray(W_qkv.T)                                # [D, 3D]
    wvT = np.ascontiguousarray(wqkvT[:, 2 * D:]).astype(bfd)             # [D, D]
    woutT = np.ascontiguousarray(W_out.T).astype(bfd)                    # [D, D]
    # fp8 main + residual of x, DR-packed [128, 2, *]: rows 256i+128t+p
    x8a = xT.astype(f8)                                                  # [B, D, N]
    x8r = (xT - x8a.astype(np.float32)).astype(f8)
    w8 = wqkvT[:, :2 * D].astype(f8)                                     # [D, 2D]

    def drpack_x(x8):
        return [np.ascontiguousarray(np.concatenate(
            [x8[:, 256 * i:256 * i + 128, :], x8[:, 256 * i + 128:256 * i + 256, :]],
            axis=2)) for i in range(NI)]                                 # [B, 128, 2N]

    xa = drpack_x(x8a)
    xr = drpack_x(x8r)
    wqf = [np.ascontiguousarray(np.concatenate(
              [w8[256 * i:256 * i + 128, :], w8[256 * i + 128:256 * i + 256, :]],
              axis=1)) for i in range(NI)]                               # [128, 4D]
    has_bqkv = bool(np.any(b_qkv != 0))

    nc = _build(has_bqkv)

    xT16 = xT.astype(bfd)
    in_maps = []
    for c in range(NCORES):
        m = {"xT": xT16[c], "wvT": wvT, "woutT": woutT}
        for i in range(NI):
            m[f"xa8_{i}"] = np.ascontiguousarray(xa[i][c])
            m[f"xr8_{i}"] = np.ascontiguousarray(xr[i][c])
            m[f"wqf{i}"] = wqf[i]
        if has_bqkv:
            m["bqkv"] = np.ascontiguousarray(b_qkv[:2 * D]).astype(np.float32)
            m["bv16"] = np.ascontiguousarray(b_qkv[2 * D:]).astype(bfd)
        in_maps.append(m)
    return nc, in_maps


def kernel(x, W_qkv, b_qkv, W_out, b_out):
    from concourse.bass_utils import run_bass_kernel_spmd

    nc, in_maps = _prepare(x, W_qkv, b_qkv, W_out, b_out)

    res = None
    for attempt in range(3):
        try:
            res = run_bass_kernel_spmd(nc, in_maps, core_ids=list(range(NCORES)))
            break
        except Exception:
            if attempt == 2:
                raise
    out = np.stack([res.results[c]["out"] for c in range(NCORES)], axis=0)
    if np.any(b_out != 0):
        out = out + b_out.astype(np.float32)
    return out


# revision 12
# speedup vs baseline: 1.0561x; 1.0561x over previous
"""Multi-head attention kernel for Trainium2, data-parallel over batch on 8 NeuronCores.

Reference computation (per batch element b of 8):
    qkv = x @ W_qkv.T + b_qkv            [1024, 2304]
    q, k, v = split(qkv)                 each [1024, 768], 12 heads x 64
    S_h = q_h @ k_h.T * d**-0.5          [1024, 1024] per head
    A_h = softmax(S_h, axis=-1)
    o_h = A_h @ v_h                      [1024, 64]
    y = concat(o) @ W_out.T + b_out      [1024, 768]

v2 strategy (one batch element per core, zero communication):
  * q/k projection in fp8 DoubleRow with an x-RESIDUAL second pass
    (x ~ x8 + fp8(x - x8)), which cancels the x-side fp8 quantization
    noise: measured rel-err 1.62e-2 in simulation vs 1.54e-2 baseline.
  * Scores S^T in fp8 DoubleRow: q/k are re-quantized to fp8 and packed
    [32 partitions x 2 k-tiles]; a DoubleRow matmul emits 512 cols in
    256 cycles (0.5 cyc/col measured), halving the S cost vs bf16.
  * A@V and both v/out projections stay bf16 (fp8 there would put ~3.6%
    noise linearly on the output).
  * V carries a leading ones column so A@V yields softmax denominators
    in PSUM partition 0; reciprocal_approx_fast reads that PSUM row
    directly (base 0 works; the known misread is at base 64).
  * Normalization is pair-batched: o^T is cast to bf16 raw, denominators
    go through recip -> DRAM -> partition-broadcast DMA, one
    tensor_tensor multiply per head PAIR produces otn[fc] (the out-proj
    stationary), saving ~30us of DVE vs per-head normalize.
  * Out-projection is staged to keep the PE fed mid-attention:
    fc0+1 after head 3, fc2+3 after head 7, fc4 after head 9 (all as
    PSUM->SBUF f32 partials), only fc5 + final add + store in the tail.
  * The PE p-state drops 2.4->1.2GHz after any idle gap and takes 3us
    of continuous work to recover, so a junk-work generator (DoubleRow
    matmuls into a dead PSUM half-bank) backstops the filler queue.
"""

import numpy as np
import ml_dtypes

B, N, D, H, HD = 8, 1024, 768, 12, 64
NCORES = 8
SCALE = float(D) ** -0.5
DC = D // 128            # 6 chunks of 128 for d=768
IC = N // 128            # 8 token chunks
KC = N // 128            # 8 key chunks
NI = 3                   # 256-deep contraction double-chunks for fp8 DR


def _build(has_bqkv: bool):
    import concourse.bass as bass
    import concourse.mybir as mybir
    import concourse.tile as tile
    from concourse import bacc

    f32 = mybir.dt.float32
    bf16 = mybir.dt.bfloat16
    fp8 = mybir.dt.float8e4
    Exp = mybir.ActivationFunctionType.Exp
    DR = mybir.MatmulPerfMode.DoubleRow
    Add = mybir.AluOpType.add
    Mult = mybir.AluOpType.mult

    nc = bacc.Bacc("TRN2", target_bir_lowering=False, debug=False,
                   num_devices=NCORES)

    xa8_ext = [nc.dram_tensor(f"xa8_{i}", [128, 2 * N], fp8, kind="ExternalInput")
               for i in range(NI)]
    xr8_ext = [nc.dram_tensor(f"xr8_{i}", [128, 2 * N], fp8, kind="ExternalInput")
               for i in range(NI)]
    wqf_ext = [nc.dram_tensor(f"wqf{i}", [128, 4 * D], fp8, kind="ExternalInput")
               for i in range(NI)]
    xT_ext = nc.dram_tensor("xT", [D, N], bf16, kind="ExternalInput")
    wvT_ext = nc.dram_tensor("wvT", [D, D], bf16, kind="ExternalInput")
    woutT_ext = nc.dram_tensor("woutT", [D, D], bf16, kind="ExternalInput")
    if has_bqkv:
        bqkv_ext = nc.dram_tensor("bqkv", [2 * D], f32, kind="ExternalInput")
        bv16_ext = nc.dram_tensor("bv16", [D], bf16, kind="ExternalInput")
    out_ext = nc.dram_tensor("out", [N, D], f32, kind="ExternalOutput")
    rdram = nc.dram_tensor("recip_scratch", [H, N], f32)

    with tile.TileContext(nc) as tc:
        with (
            tc.tile_pool(name="w", bufs=1) as wpool,
            tc.tile_pool(name="act", bufs=1) as apool,
            tc.tile_pool(name="es", bufs=7) as espool,
            tc.tile_pool(name="rows", bufs=2) as rowpool,
            tc.tile_pool(name="bc", bufs=2) as bcpool,
            tc.tile_pool(name="y", bufs=3) as ypool,
            tc.tile_pool(name="ps", bufs=1, space="PSUM") as pspool,
        ):
            # ---- input DMAs: qk-proj operands first for earliest PE start ----
            xa8 = [wpool.tile([128, 2 * N], fp8, tag=f"xa8_{i}", name=f"xa8_{i}") for i in range(NI)]
            xr8 = [wpool.tile([128, 2 * N], fp8, tag=f"xr8_{i}", name=f"xr8_{i}") for i in range(NI)]
            wqf = [wpool.tile([128, 4 * D], fp8, tag=f"wqf{i}", name=f"wqf{i}") for i in range(NI)]
            xT = [wpool.tile([128, N], bf16, tag=f"xT{i}", name=f"xT{i}") for i in range(DC)]
            wv = [wpool.tile([128, D], bf16, tag=f"wv{i}", name=f"wv{i}") for i in range(DC)]
            wo = [wpool.tile([128, D], bf16, tag=f"wo{i}", name=f"wo{i}") for i in range(DC)]
            # queue plan: sync = xa/wqf then xT then wo (bulk input stream);
            # gpsimd = xr only, so it is free for the qk pack DMAs at ~6us;
            # wv follows the packs on gpsimd (v chunks start ~10us).
            for i in range(NI):
                nc.sync.dma_start(out=xa8[i][:], in_=xa8_ext[i][:, :])
                nc.sync.dma_start(out=wqf[i][:], in_=wqf_ext[i][:, :])
            for i in range(NI):
                nc.gpsimd.dma_start(out=xr8[i][:], in_=xr8_ext[i][:, :])
            for dc in range(DC):
                nc.sync.dma_start(out=xT[dc][:], in_=xT_ext[dc * 128:(dc + 1) * 128, :])

            if has_bqkv:
                bqk_t = wpool.tile([128, 2 * DC], f32, tag="bqk")
                for jc in range(2 * DC):
                    nc.sync.dma_start(
                        out=bqk_t[:, jc:jc + 1],
                        in_=bqkv_ext[jc * 128:(jc + 1) * 128][:, None])
                bv_t = wpool.tile([1, D], bf16, tag="bv")
                nc.sync.dma_start(out=bv_t[:], in_=bv16_ext[:][None, :])
                ones_t = wpool.tile([1, 128], bf16, tag="ones")
                nc.vector.memset(ones_t[:], 1.0)

            xa3 = [t.rearrange("p (two n) -> p two n", two=2) for t in xa8]
            xr3 = [t.rearrange("p (two n) -> p two n", two=2) for t in xr8]
            wqf3 = [t.rearrange("p (two n) -> p two n", two=2) for t in wqf]

            # ---- q/k fp8 DR-packed tiles: chunk jc (0..5 q, 6..11 k) ----
            # feature 64*hh + 32*t + p  ->  partition 64*hh + p, k-tile t
            qk8 = [apool.tile([128, 2, N], fp8, tag=f"qk8_{j}", name=f"qk8_{j}")
                   for j in range(2 * DC)]
            f8s = [apool.tile([128, N], fp8, tag=f"f8s{i}", name=f"f8s{i}")
                   for i in range(2)]

            # PSUM budget (8 banks): A,B = sps double-buffer (2+2), C = ot (2),
            # D = two [128,512] one-bank tiles for fillers + junk.
            def big_ps(tag, name):
                return pspool.tile([128, N], f32, tag=tag, name=name)

            def half_ps(name):
                return pspool.tile([128, 512], f32, tag="D", bufs=2, name=name)

            def qk_mms(ps, jc, s0, cols):
                """fp8 DR passes (main + x-residual) for features chunk jc.

                ps is a [128, cols] PSUM region; matmuls are emitted in
                512-wide pieces (PSUM bank limit)."""
                for i in range(NI):
                    for x3 in (xa3, xr3):
                        for ih in range(cols // 512):
                            yield nc.tensor.matmul(
                                ps[:, ih * 512:(ih + 1) * 512],
                                wqf3[i][:, :, jc * 128:(jc + 1) * 128],
                                x3[i][:, :, s0 + ih * 512:s0 + (ih + 1) * 512],
                                start=(i == 0 and x3 is xa3),
                                stop=(i == NI - 1 and x3 is xr3),
                                perf_mode=DR)

            def qk_pack(jc, ps, s, e, eng):
                """Cast PSUM f32 -> fp8 and shuffle into DR layout."""
                fs = f8s[jc % 2]
                if has_bqkv:
                    nc.vector.tensor_scalar_add(fs[:, s:e], ps[:, 0:e - s],
                                                bqk_t[:, jc:jc + 1])
                else:
                    nc.vector.tensor_copy(fs[:, s:e], ps[:, 0:e - s])
                t = qk8[jc]
                eng.dma_start(out=t[0:32, 0, s:e], in_=fs[0:32, s:e])
                eng.dma_start(out=t[0:32, 1, s:e], in_=fs[32:64, s:e])
                eng.dma_start(out=t[64:96, 0, s:e], in_=fs[64:96, s:e])
                eng.dma_start(out=t[64:96, 1, s:e], in_=fs[96:128, s:e])

            def qk_chunk_full(jc, tag, eng):
                ps = big_ps(tag, f"qkps{jc}")
                for _ in qk_mms(ps, jc, 0, N):
                    pass
                qk_pack(jc, ps, 0, N, eng)

            def gen_qk_chunk(jc):
                """Filler generator: one chunk as 2 half-bank units."""
                for ih in range(2):
                    ps = half_ps(f"qkh{jc}_{ih}")
                    yield from qk_mms(ps, jc, ih * 512, 512)
                    qk_pack(jc, ps, ih * 512, (ih + 1) * 512,
                            nc.sync if ih == 0 else nc.gpsimd)

            # ---- v : [1024 tokens, 12 heads x (64+1)] with trailing ones col ----
            v = [apool.tile([128, H, HD + 1], bf16, tag=f"v{i}", name=f"v{i}") for i in range(IC)]
            for ic in range(IC):
                nc.vector.memset(v[ic][:, :, HD:HD + 1], 1.0)
            vsplits = [(0, 512), (512, 768)]

            def v_mms(ps, ic, s, e):
                if has_bqkv:
                    yield nc.tensor.matmul(ps[:, 0:e - s], ones_t[:],
                                           bv_t[:, s:e], start=True, stop=False)
                for dc in range(DC):
                    yield nc.tensor.matmul(
                        ps[:, 0:e - s],
                        xT[dc][:, ic * 128:(ic + 1) * 128],
                        wv[dc][:, s:e],
                        start=(dc == 0 and not has_bqkv), stop=(dc == DC - 1))

            def v_store(ic, ps, s, e):
                h0, h1 = s // HD, e // HD
                nc.vector.tensor_copy(
                    v[ic][:, h0:h1, 0:HD],
                    ps[:, 0:e - s].rearrange("p (h e) -> p h e", h=h1 - h0))

            def v_chunk(ic, tag):
                ps = big_ps(tag, f"vps{ic}")
                for s, e in vsplits:
                    for _ in v_mms(ps[:, s:], ic, s, e):
                        pass
                for s, e in vsplits:
                    v_store(ic, ps[:, s:], s, e)

            def gen_v_chunk(ic):
                for s, e in vsplits:
                    ps = half_ps(f"vh{ic}_{s}")
                    yield from v_mms(ps, ic, s, e)
                    v_store(ic, ps, s, e)

            # ---- attention: flat software pipeline over (head, kc) ----
            otu16 = {}

            fillers = []

            def fill(n):
                while n > 0 and fillers:
                    try:
                        next(fillers[0])
                        n -= 1
                    except StopIteration:
                        fillers.pop(0)

            def smm(h, kc, sps):
                qt, kt = qk8[h // 2], qk8[DC + h // 2]
                p0 = 64 * (h % 2)
                for ih in range(2):
                    nc.tensor.matmul(
                        sps[:, ih * 512:(ih + 1) * 512],
                        kt[p0:p0 + 32, :, kc * 128:(kc + 1) * 128],
                        qt[p0:p0 + 32, :, ih * 512:(ih + 1) * 512],
                        start=True, stop=True, perf_mode=DR)

            def avmm(h, kc, et, ot):
                for ih in range(2):
                    nc.tensor.matmul(
                        ot[0:HD + 1, ih * 512:(ih + 1) * 512],
                        v[kc][:, h, :],
                        et[:, ih * 512:(ih + 1) * 512],
                        start=(kc == 0), stop=(kc == KC - 1))

            def head_epilogue(h, ot):
                # denominator staging first (it gates normalize); the custom
                # recip op misreads PSUM rows at partition base 64, so stage
                # to a base-0 SBUF row first.
                t = h // 2
                drow = rowpool.tile([1, N], f32, tag="drow", bufs=2, name=f"drow{h}")
                nc.vector.tensor_copy(drow[:], ot[HD:HD + 1, :])
                rc32 = rowpool.tile([1, N], f32, tag="rc32", bufs=2, name=f"rc32_{h}")
                nc.vector.reciprocal_approx_fast(rc32[:], drow[:])
                eng = nc.sync if h % 2 == 0 else nc.gpsimd
                eng.dma_start(out=rdram[h:h + 1, :], in_=rc32[:])
                if h % 2 == 0:
                    otu16[t] = rowpool.tile([128, N], bf16, tag="otu16", bufs=2,
                                            name=f"otu16_{t}")
                with nc.allow_low_precision(reason="bf16 raw o; 2e-2 gate"):
                    nc.vector.tensor_copy(
                        otu16[t][64 * (h % 2):64 * (h % 2) + 64, :],
                        ot[0:HD, :])
                bch = bcpool.tile([128, N], f32, tag="bc", bufs=2, name=f"bc{t}") \
                    if h % 2 == 0 else bcs[t]
                if h % 2 == 0:
                    bcs[t] = bch
                eng.dma_start(
                    out=bch[64 * (h % 2):64 * (h % 2) + 64, :],
                    in_=rdram[h:h + 1, :].to_broadcast((64, N)))

            bcs = {}

            def normalize_pair(t):
                with nc.allow_low_precision(reason="bf16 normalized o; 2e-2 gate"):
                    nc.vector.tensor_tensor(
                        out=otn[t][:], in0=otu16.pop(t)[:], in1=bcs.pop(t)[:],
                        op=Mult)

            def normalize_half(h):
                t, i = h // 2, h % 2
                with nc.allow_low_precision(reason="bf16 normalized o; 2e-2 gate"):
                    nc.vector.tensor_tensor(
                        out=otn[t][64 * i:64 * i + 64, :],
                        in0=otu16[t][64 * i:64 * i + 64, :],
                        in1=bcs[t][64 * i:64 * i + 64, :], op=Mult)
                if i == 1:
                    otu16.pop(t)
                    bcs.pop(t)

            # ---- output projection ----
            # ypart[ic] accumulates fc stages in f32 SBUF via filler units;
            # tail adds fc5 from PSUM.
            otn = [apool.tile([128, N], bf16, tag=f"otn{t}", name=f"otn{t}") for t in range(DC)]
            ypart = [apool.tile([128, D], f32, tag=f"yp{ic}", name=f"yp{ic}") for ic in range(IC)]

            def gen_outproj_partial(ic):
                """fc 0..3 partial into f32 SBUF (fillers after head 7)."""
                for s, e in vsplits:
                    ps = half_ps(f"yh{ic}_{s}")
                    for fc in range(4):
                        yield nc.tensor.matmul(
                            ps[:, 0:e - s],
                            otn[fc][:, ic * 128:(ic + 1) * 128],
                            wo[fc][:, s:e],
                            start=(fc == 0), stop=(fc == 3))
                    nc.vector.tensor_copy(ypart[ic][:, s:e], ps[:, 0:e - s])

            def outproj_tail_mm(ic, ps, fc):
                for s, e in vsplits:
                    nc.tensor.matmul(
                        ps[:, s:e],
                        otn[fc][:, ic * 128:(ic + 1) * 128],
                        wo[fc][:, s:e],
                        start=(fc == 4), stop=(fc == 5))

            def outproj_finish(ic, ps):
                ysb = ypool.tile([128, D], f32, tag="y", name=f"y{ic}")
                nc.vector.tensor_tensor(
                    out=ysb[:], in0=ps[:, 0:D], in1=ypart[ic][:], op=Add)
                eng = nc.sync if ic % 2 == 0 else nc.gpsimd
                eng.dma_start(out=out_ext[ic * 128:(ic + 1) * 128, :], in_=ysb[:])

            # ---- phase A ----
            tags = ["A", "B", "C"]
            ets = {}
            ots = {}
            LAG = 3   # steps between S emission and its A@V consumption

            def emit_s(step):
                h, kc = divmod(step, KC)
                sps = big_ps("A" if step % 2 == 0 else "B", f"sps{h}_{kc}")
                smm(h, kc, sps)
                et = espool.tile([128, N], bf16, tag="es", name=f"es{h}_{kc}")
                nc.scalar.activation(et[:], sps[:], Exp, scale=SCALE)
                ets[step] = et

            def consume(gen):
                for _ in gen:
                    pass

            qk_chunk_full(0, "A", nc.gpsimd)
            qk_chunk_full(DC, "B", nc.gpsimd)
            # bulk v/out weights follow the time-critical packs on the queues
            for dc in range(DC):
                nc.gpsimd.dma_start(out=wv[dc][:], in_=wvT_ext[dc * 128:(dc + 1) * 128, :])
            for dc in range(DC):
                nc.sync.dma_start(out=wo[dc][:], in_=woutT_ext[dc * 128:(dc + 1) * 128, :])
            # S(0,*) interleaved with the remaining head-1/2 qk chunks and v;
            # C and D psum regions alternate so WAR waits are pre-satisfied.
            emit_s(0)
            consume(gen_qk_chunk(1))
            emit_s(1)
            consume(gen_qk_chunk(DC + 1))
            emit_s(2)
            v_chunk(0, "C")
            emit_s(3)
            consume(gen_v_chunk(1))
            emit_s(4)
            v_chunk(2, "C")
            emit_s(5)
            consume(gen_v_chunk(3))
            emit_s(6)
            v_chunk(4, "C")
            emit_s(7)
            consume(gen_v_chunk(5))

            # filler queue in deadline order.
            fillers.append(gen_v_chunk(6))
            fillers.append(gen_v_chunk(7))
            for jc in [2, DC + 2, 3, DC + 3, 4, DC + 4, 5, DC + 5]:
                fillers.append(gen_qk_chunk(jc))

            emitted = 8   # S(0,*) issued during phase A
            for step in range(H * KC + LAG):
                if step < H * KC and step >= emitted:
                    emit_s(step)
                    emitted = step + 1
                boundary = False
                a = step - LAG
                if a >= 0:
                    hp, kcp = divmod(a, KC)
                    if kcp == 0:
                        ots[hp] = big_ps("C", f"ot{hp}")
                    avmm(hp, kcp, ets.pop(a), ots[hp])
                    if kcp == KC - 1:
                        boundary = True
                        head_epilogue(hp, ots.pop(hp))
                        if hp % 2 == 1 and hp < 10:
                            normalize_pair(hp // 2)
                        elif hp >= 10:
                            normalize_half(hp)   # split last pair for the tail
                        if hp == 7:
                            for ic in range(IC - 2):
                                fillers.append(gen_outproj_partial(ic))
                        # pre-emit the next step's S so ACT keeps cadence
                        if step + 1 < H * KC and step + 1 >= emitted:
                            emit_s(step + 1)
                            emitted = step + 2
                fill(10 if boundary else (4 if step % 2 == 0 else 3))

            # tail: fc4 (ready since head 9) fans out while the last pair's
            # normalize chain completes; fc5 + final add + store per ic.
            fillers.append(gen_outproj_partial(IC - 2))
            fillers.append(gen_outproj_partial(IC - 1))
            fill(10 ** 9)
            tps = {}
            for ic in range(IC):
                tps[ic] = big_ps(tags[ic % 3], f"yt{ic}")
                outproj_tail_mm(ic, tps[ic], 4)
                if ic >= 2:
                    j = ic - 2
                    outproj_tail_mm(j, tps[j], 5)
                    outproj_finish(j, tps.pop(j))
            for ic in (IC - 2, IC - 1):
                outproj_tail_mm(ic, tps[ic], 5)
                outproj_finish(ic, tps.pop(ic))

    nc.compile()
    return nc


def _prepare(x, W_qkv, b_qkv, W_out, b_out):
    """Build the compiled graph and per-core input maps."""
    bfd = ml_dtypes.bfloat16
    f8 = ml_dtypes.float8_e4m3
    xT = np.ascontiguousarray(np.transpose(x, (0, 2, 1)))                # [B, D, N] f32
    wqkvT = np.ascontiguousarray(W_qkv.T)                                # [D, 3D]
    wvT = np.ascontiguousarray(wqkvT[:, 2 * D:]).astype(bfd)             # [D, D]
    woutT = np.ascontiguousarray(W_out.T).astype(bfd)                    # [D, D]
    # fp8 main + residual of x, DR-packed [128, 2, *]: rows 256i+128t+p
    x8a = xT.astype(f8)                                                  # [B, D, N]
    x8r = (xT - x8a.astype(np.float32)).astype(f8)
    w8 = wqkvT[:, :2 * D].astype(f8)                                     # [D, 2D]

    def drpack_x(x8):
        return [np.ascontiguousarray(np.concatenate(
            [x8[:, 256 * i:256 * i + 128, :], x8[:, 256 * i + 128:256 * i + 256, :]],
            axis=2)) for i in range(NI)]                                 # [B, 128, 2N]

    xa = drpack_x(x8a)
    xr = drpack_x(x8r)
    wqf = [np.ascontiguousarray(np.concatenate(
              [w8[256 * i:256 * i + 128, :], w8[256 * i + 128:256 * i + 256, :]],
              axis=1)) for i in range(NI)]                               # [128, 4D]
    has_bqkv = bool(np.any(b_qkv != 0))

    nc = _build(has_bqkv)

    xT16 = xT.astype(bfd)
    in_maps = []
    for c in range(NCORES):
        m = {"xT": xT16[c], "wvT": wvT, "woutT": woutT}
        for i in range(NI):
            m[f"xa8_{i}"] = np.ascontiguousarray(xa[i][c])
            m[f"xr8_{i}"] = np.ascontiguousarray(xr[i][c])
            m[f"wqf{i}"] = wqf[i]
        if has_bqkv:
            m["bqkv"] = np.ascontiguousarray(b_qkv[:2 * D]).astype(np.float32)
            m["bv16"] = np.ascontiguousarray(b_qkv[2 * D:]).astype(bfd)
        in_maps.append(m)
    return nc, in_maps


def kernel(x, W_qkv, b_qkv, W_out, b_out):
    from concourse.bass_utils import run_bass_kernel_spmd

    nc, in_maps = _prepare(x, W_qkv, b_qkv, W_out, b_out)

    res = None
    for attempt in range(3):
        try:
            res = run_bass_kernel_spmd(nc, in_maps, core_ids=list(range(NCORES)))
            break
        except Exception:
            if attempt == 2:
                raise
    out = np.stack([res.results[c]["out"] for c in range(NCORES)], axis=0)
    if np.any(b_out != 0):
        out = out + b_out.astype(np.float32)
    return out


# revision 17
# speedup vs baseline: 1.1232x; 1.0635x over previous
"""Multi-head attention kernel for Trainium2, data-parallel over batch on 8 NeuronCores.

Reference computation (per batch element b of 8):
    qkv = x @ W_qkv.T + b_qkv            [1024, 2304]
    q, k, v = split(qkv)                 each [1024, 768], 12 heads x 64
    S_h = q_h @ k_h.T * d**-0.5          [1024, 1024] per head
    A_h = softmax(S_h, axis=-1)
    o_h = A_h @ v_h                      [1024, 64]
    y = concat(o) @ W_out.T + b_out      [1024, 768]

v4 strategy (one batch element per core, zero communication):
  * q/k projection in fp8 DoubleRow with an x-RESIDUAL second pass
    (x ~ x8 + fp8(x - x8)), cancelling the x-side fp8 quantization noise.
  * Scores S^T in fp8 DoubleRow: q/k re-quantized to fp8; a DoubleRow
    matmul emits 512 cols in 256 cycles. W_qkv columns are permuted on
    the host ([A0-31|B0-31|A32-63|B32-63] per 128-chunk) so the DR
    [32p x 2slot] packing is ONE cast + ONE 64-partition self-DMA.
  * A@V and the v/out projections stay bf16 (fp8 there is ~3.6% output
    noise). V carries a trailing ones column -> softmax denominators.
  * Normalization pair-batched: raw o^T cast to bf16, denominator row
    staged to SBUF, reciprocal_approx_fast, partition-broadcast DMA,
    one tensor_tensor multiply per head pair (split per-head for the
    last pair to shorten the tail).
  * DMA issue costs ~600ns of queue time each, so inputs are 6
    consolidated transfers; scalar queue carries ONLY the exp
    activations; outputs fan out over 4 queues.
  * The PE p-state drops 2.4->1.2GHz after any idle gap (several us of
    continuous work to recover) and semaphores are coarse, so: S for
    step+1 is emitted BEFORE each head-boundary DVE chain, A@V lags S
    by LAG steps so its waits are pre-satisfied, and a filler queue
    (deferred qk/v chunks, then out-proj fc0..3 partials) keeps the PE
    dense through the attention steady state.
"""

import numpy as np
import ml_dtypes

B, N, D, H, HD = 8, 1024, 768, 12, 64
NCORES = 8
SCALE = float(D) ** -0.5
DC = D // 128            # 6 chunks of 128 for d=768
IC = N // 128            # 8 token chunks
KC = N // 128            # 8 key chunks
NI = 3                   # 256-deep contraction double-chunks for fp8 DR


def _build(has_bqkv: bool):
    import concourse.bass as bass
    import concourse.mybir as mybir
    import concourse.tile as tile
    from concourse import bacc

    f32 = mybir.dt.float32
    bf16 = mybir.dt.bfloat16
    fp8 = mybir.dt.float8e4
    Exp = mybir.ActivationFunctionType.Exp
    DR = mybir.MatmulPerfMode.DoubleRow
    Add = mybir.AluOpType.add
    Mult = mybir.AluOpType.mult

    nc = bacc.Bacc("TRN2", target_bir_lowering=False, debug=False,
                   num_devices=NCORES)

    xa_ext = nc.dram_tensor("xa", [128, NI * 2 * N], fp8, kind="ExternalInput")
    xr_ext = nc.dram_tensor("xr", [128, NI * 2 * N], fp8, kind="ExternalInput")
    wqf_ext = nc.dram_tensor("wqf", [128, NI * 4 * D], fp8, kind="ExternalInput")
    xT_ext = nc.dram_tensor("xT", [128, DC * N], bf16, kind="ExternalInput")
    wv_ext = nc.dram_tensor("wv", [128, DC * D], bf16, kind="ExternalInput")
    wo_ext = nc.dram_tensor("wo", [128, DC * D], bf16, kind="ExternalInput")
    if has_bqkv:
        bqkv_ext = nc.dram_tensor("bqkv", [2 * D], f32, kind="ExternalInput")
        bv16_ext = nc.dram_tensor("bv16", [D], bf16, kind="ExternalInput")
    out_ext = nc.dram_tensor("out", [N, D], f32, kind="ExternalOutput")
    rdram = nc.dram_tensor("recip_scratch", [H, N], f32)

    with tile.TileContext(nc) as tc:
        with (
            tc.tile_pool(name="w", bufs=1) as wpool,
            tc.tile_pool(name="act", bufs=1) as apool,
            tc.tile_pool(name="es", bufs=8) as espool,
            tc.tile_pool(name="rows", bufs=2) as rowpool,
            tc.tile_pool(name="bc", bufs=2) as bcpool,
            tc.tile_pool(name="y", bufs=3) as ypool,
            tc.tile_pool(name="ps", bufs=1, space="PSUM") as pspool,
        ):
            # ---- consolidated input DMAs (issue cost ~600ns each) ----
            xa_t = wpool.tile([128, NI * 2 * N], fp8, tag="xa")
            xr_t = wpool.tile([128, NI * 2 * N], fp8, tag="xr")
            wqf_t = wpool.tile([128, NI * 4 * D], fp8, tag="wqf")
            xT_t = wpool.tile([128, DC * N], bf16, tag="xT")
            wv_t = wpool.tile([128, DC * D], bf16, tag="wv")
            wo_t = wpool.tile([128, DC * D], bf16, tag="wo")
            nc.sync.dma_start(out=xa_t[:], in_=xa_ext[:, :])
            nc.sync.dma_start(out=wqf_t[:], in_=wqf_ext[:, :])
            nc.gpsimd.dma_start(out=xr_t[:], in_=xr_ext[:, :])
            nc.gpsimd.dma_start(out=wv_t[:], in_=wv_ext[:, :])
            nc.sync.dma_start(out=xT_t[:], in_=xT_ext[:, :])
            nc.sync.dma_start(out=wo_t[:], in_=wo_ext[:, :])

            if has_bqkv:
                bqk_t = wpool.tile([128, 2 * DC], f32, tag="bqk")
                for jc in range(2 * DC):
                    nc.sync.dma_start(
                        out=bqk_t[:, jc:jc + 1],
                        in_=bqkv_ext[jc * 128:(jc + 1) * 128][:, None])
                bv_t = wpool.tile([1, D], bf16, tag="bv")
                nc.sync.dma_start(out=bv_t[:], in_=bv16_ext[:][None, :])
                ones_t = wpool.tile([1, 128], bf16, tag="ones")
                nc.vector.memset(ones_t[:], 1.0)

            xa4 = xa_t.rearrange("p (i two n) -> p i two n", i=NI, two=2)
            xr4 = xr_t.rearrange("p (i two n) -> p i two n", i=NI, two=2)
            wqf4 = wqf_t.rearrange("p (i two n) -> p i two n", i=NI, two=2)
            xa3 = [xa4[:, i] for i in range(NI)]
            xr3 = [xr4[:, i] for i in range(NI)]
            wqf3 = [wqf4[:, i] for i in range(NI)]
            xTv = xT_t.rearrange("p (c n) -> p c n", c=DC)
            wvv = wv_t.rearrange("p (c n) -> p c n", c=DC)
            wov = wo_t.rearrange("p (c n) -> p c n", c=DC)
            xT = [xTv[:, c] for c in range(DC)]
            wv = [wvv[:, c] for c in range(DC)]
            wo = [wov[:, c] for c in range(DC)]

            # ---- q/k fp8 DR tiles: chunk jc (0..5 q, 6..11 k) ----
            # host W-permutation puts [A0-31|B0-31|A32-63|B32-63] on the
            # chunk's partitions, so slot1 = one partition-shift self-DMA.
            qk8 = [apool.tile([128, 2, N], fp8, tag=f"qk8_{j}", name=f"qk8_{j}")
                   for j in range(2 * DC)]

            # PSUM budget (8 banks): A,B = sps double-buffer (2+2), C = ot (2),
            # D = two [128,512] one-bank tiles for fillers.
            def big_ps(tag, name):
                return pspool.tile([128, N], f32, tag=tag, name=name)

            def half_ps(name):
                return pspool.tile([128, 512], f32, tag="D", bufs=2, name=name)

            def qk_mms(ps, jc, s0, cols):
                """fp8 DR passes (main + x-residual) for features chunk jc,
                512-wide pieces (PSUM bank limit)."""
                for i in range(NI):
                    for x3 in (xa3, xr3):
                        for ih in range(cols // 512):
                            yield nc.tensor.matmul(
                                ps[:, ih * 512:(ih + 1) * 512],
                                wqf3[i][:, :, jc * 128:(jc + 1) * 128],
                                x3[i][:, :, s0 + ih * 512:s0 + (ih + 1) * 512],
                                start=(i == 0 and x3 is xa3),
                                stop=(i == NI - 1 and x3 is xr3),
                                perf_mode=DR)

            def qk_pack(jc, ps, s, e, eng):
                """Cast PSUM f32 -> fp8 slot0, shift upper half to slot1."""
                t = qk8[jc]
                if has_bqkv:
                    nc.vector.tensor_scalar_add(t[:, 0, s:e], ps[:, 0:e - s],
                                                bqk_t[:, jc:jc + 1])
                else:
                    nc.vector.tensor_copy(t[:, 0, s:e], ps[:, 0:e - s])
                eng.dma_start(out=t[0:64, 1, s:e], in_=t[64:128, 0, s:e])

            def qk_chunk_full(jc, tag, eng):
                ps = big_ps(tag, f"qkps{jc}")
                for _ in qk_mms(ps, jc, 0, N):
                    pass
                qk_pack(jc, ps, 0, N, eng)

            def gen_qk_chunk(jc):
                """Filler generator: one chunk as 2 half-bank units."""
                for ih in range(2):
                    ps = half_ps(f"qkh{jc}_{ih}")
                    yield from qk_mms(ps, jc, ih * 512, 512)
                    qk_pack(jc, ps, ih * 512, (ih + 1) * 512,
                            nc.sync if ih == 0 else nc.gpsimd)

            # ---- v : [1024 tokens, 12 heads x (64+1)] with trailing ones ----
            v = [apool.tile([128, H, HD + 1], bf16, tag=f"v{i}", name=f"v{i}") for i in range(IC)]
            vsplits = [(0, 512), (512, 768)]

            def v_mms(ps, ic, s, e):
                if has_bqkv:
                    yield nc.tensor.matmul(ps[:, 0:e - s], ones_t[:],
                                           bv_t[:, s:e], start=True, stop=False)
                for dc in range(DC):
                    yield nc.tensor.matmul(
                        ps[:, 0:e - s],
                        xT[dc][:, ic * 128:(ic + 1) * 128],
                        wv[dc][:, s:e],
                        start=(dc == 0 and not has_bqkv), stop=(dc == DC - 1))

            def v_store(ic, ps, s, e):
                h0, h1 = s // HD, e // HD
                nc.vector.tensor_copy(
                    v[ic][:, h0:h1, 0:HD],
                    ps[:, 0:e - s].rearrange("p (h e) -> p h e", h=h1 - h0))

            def v_chunk(ic, tag):
                ps = big_ps(tag, f"vps{ic}")
                for s, e in vsplits:
                    for _ in v_mms(ps[:, s:], ic, s, e):
                        pass
                for s, e in vsplits:
                    v_store(ic, ps[:, s:], s, e)

            def gen_v_chunk(ic):
                for s, e in vsplits:
                    ps = half_ps(f"vh{ic}_{s}")
                    yield from v_mms(ps, ic, s, e)
                    v_store(ic, ps, s, e)

            # ---- attention: flat software pipeline over (head, kc) ----
            otu16 = {}
            bcs = {}
            fillers = []

            def fill(n):
                while n > 0 and fillers:
                    try:
                        next(fillers[0])
                        n -= 1
                    except StopIteration:
                        fillers.pop(0)

            def smm(h, kc, sps):
                qt, kt = qk8[h // 2], qk8[DC + h // 2]
                p0 = 32 * (h % 2)
                for ih in range(2):
                    nc.tensor.matmul(
                        sps[:, ih * 512:(ih + 1) * 512],
                        kt[p0:p0 + 32, :, kc * 128:(kc + 1) * 128],
                        qt[p0:p0 + 32, :, ih * 512:(ih + 1) * 512],
                        start=True, stop=True, perf_mode=DR)

            def avmm(h, kc, et, ot):
                for ih in range(2):
                    nc.tensor.matmul(
                        ot[0:HD + 1, ih * 512:(ih + 1) * 512],
                        v[kc][:, h, :],
                        et[:, ih * 512:(ih + 1) * 512],
                        start=(kc == 0), stop=(kc == KC - 1))

            def head_epilogue(h, ot):
                # denominator staging first (it gates normalize); recip
                # misreads PSUM rows at partition base 64, so stage to a
                # base-0 SBUF row first.
                t = h // 2
                drow = rowpool.tile([1, N], f32, tag="drow", bufs=2, name=f"drow{h}")
                nc.vector.tensor_copy(drow[:], ot[HD:HD + 1, :])
                rc32 = rowpool.tile([1, N], f32, tag="rc32", bufs=2, name=f"rc32_{h}")
                nc.vector.reciprocal_approx_fast(rc32[:], drow[:])
                if h % 2 == 0:
                    otu16[t] = rowpool.tile([128, N], bf16, tag="otu16", bufs=2,
                                            name=f"otu16_{t}")
                    bcs[t] = bcpool.tile([128, N], f32, tag="bc", bufs=2,
                                         name=f"bc{t}")
                with nc.allow_low_precision(reason="bf16 raw o; 2e-2 gate"):
                    nc.vector.tensor_copy(
                        otu16[t][64 * (h % 2):64 * (h % 2) + 64, :],
                        ot[0:HD, :])
                eng = nc.sync if h % 2 == 0 else nc.gpsimd
                # partition-broadcast via DRAM (SBUF source can't stride-0)
                eng.dma_start(out=rdram[h:h + 1, :], in_=rc32[:])
                eng.dma_start(
                    out=bcs[t][64 * (h % 2):64 * (h % 2) + 64, :],
                    in_=rdram[h:h + 1, :].to_broadcast((64, N)))

            def normalize_pair(t):
                with nc.allow_low_precision(reason="bf16 normalized o; 2e-2 gate"):
                    nc.vector.tensor_tensor(
                        out=otn[t][:], in0=otu16.pop(t)[:], in1=bcs.pop(t)[:],
                        op=Mult)

            def normalize_half(h):
                t, i = h // 2, h % 2
                with nc.allow_low_precision(reason="bf16 normalized o; 2e-2 gate"):
                    nc.vector.tensor_tensor(
                        out=otn[t][64 * i:64 * i + 64, :],
                        in0=otu16[t][64 * i:64 * i + 64, :],
                        in1=bcs[t][64 * i:64 * i + 64, :], op=Mult)
                if i == 1:
                    otu16.pop(t)
                    bcs.pop(t)

            # ---- output projection ----
            otn = [apool.tile([128, N], bf16, tag=f"otn{t}", name=f"otn{t}") for t in range(DC)]
            ypart = [apool.tile([128, D], f32, tag=f"yp{ic}", name=f"yp{ic}") for ic in range(IC)]

            def gen_outproj_partial(ic):
                """fc 0..3 partial into f32 SBUF (fillers after head 7)."""
                for s, e in vsplits:
                    ps = half_ps(f"yh{ic}_{s}")
                    for fc in range(4):
                        yield nc.tensor.matmul(
                            ps[:, 0:e - s],
                            otn[fc][:, ic * 128:(ic + 1) * 128],
                            wo[fc][:, s:e],
                            start=(fc == 0), stop=(fc == 3))
                    nc.vector.tensor_copy(ypart[ic][:, s:e], ps[:, 0:e - s])

            def outproj_tail_mm(ic, ps, fc):
                for s, e in vsplits:
                    nc.tensor.matmul(
                        ps[:, s:e],
                        otn[fc][:, ic * 128:(ic + 1) * 128],
                        wo[fc][:, s:e],
                        start=(fc == 4), stop=(fc == 5))

            # scalar is free once the last exp retires (tail-only use)
            out_engs = [nc.sync, nc.gpsimd, nc.scalar]

            def outproj_finish(ic, ps):
                ysb = ypool.tile([128, D], f32, tag="y", name=f"y{ic}")
                nc.vector.tensor_tensor(
                    out=ysb[:], in0=ps[:, 0:D], in1=ypart[ic][:], op=Add)
                out_engs[ic % 3].dma_start(
                    out=out_ext[ic * 128:(ic + 1) * 128, :], in_=ysb[:])

            # ---- phase A ----
            tags = ["A", "B", "C"]
            ets = {}
            ots = {}
            tps = {}
            LAG = 3   # steps between S emission and its A@V consumption

            def emit_s(step):
                h, kc = divmod(step, KC)
                sps = big_ps("A" if step % 2 == 0 else "B", f"sps{h}_{kc}")
                smm(h, kc, sps)
                et = espool.tile([128, N], bf16, tag="es", name=f"es{h}_{kc}")
                nc.scalar.activation(et[:], sps[:], Exp, scale=SCALE)
                ets[step] = et

            def consume(gen):
                for _ in gen:
                    pass

            qk_chunk_full(0, "A", nc.gpsimd)
            qk_chunk_full(DC, "B", nc.gpsimd)
            for ic in range(IC):
                nc.vector.memset(v[ic][:, :, HD:HD + 1], 1.0)
            # S(0,*) interleaved with the head-1/2 qk chunks and v chunks;
            # C and D psum regions alternate so WAR waits are pre-satisfied.
            emit_s(0)
            consume(gen_qk_chunk(1))
            emit_s(1)
            consume(gen_qk_chunk(DC + 1))
            emit_s(2)
            v_chunk(0, "C")
            emit_s(3)
            consume(gen_v_chunk(1))
            emit_s(4)
            v_chunk(2, "C")
            emit_s(5)
            consume(gen_v_chunk(3))
            emit_s(6)
            v_chunk(4, "C")
            emit_s(7)
            consume(gen_v_chunk(5))

            # filler queue in deadline order.
            fillers.append(gen_v_chunk(6))
            fillers.append(gen_v_chunk(7))
            for jc in [2, DC + 2, 3, DC + 3, 4, DC + 4, 5, DC + 5]:
                fillers.append(gen_qk_chunk(jc))

            emitted = 8   # S(0,*) issued during phase A
            for step in range(H * KC + LAG):
                if step < H * KC and step >= emitted:
                    emit_s(step)
                    emitted = step + 1
                boundary = False
                a = step - LAG
                if a >= 0:
                    hp, kcp = divmod(a, KC)
                    if kcp == 0:
                        ots[hp] = big_ps("C", f"ot{hp}")
                    avmm(hp, kcp, ets.pop(a), ots[hp])
                    if kcp == KC - 1:
                        boundary = True
                        # next S first: with coarse semaphores, anything
                        # emitted after the DVE chain tends to wait on it.
                        if step + 1 < H * KC and step + 1 >= emitted:
                            emit_s(step + 1)
                            emitted = step + 2
                        if hp == 11:
                            # fc4 for two tiles ahead of the last DVE chain
                            for ic in (0, 1):
                                tps[ic] = big_ps(tags[ic], f"yt{ic}")
                                outproj_tail_mm(ic, tps[ic], 4)
                        fill(3)
                        head_epilogue(hp, ots.pop(hp))
                        if hp % 2 == 1 and hp < 10:
                            normalize_pair(hp // 2)
                        elif hp >= 10:
                            normalize_half(hp)
                        if hp == 7:
                            for ic in range(IC):
                                fillers.append(gen_outproj_partial(ic))
                fill(8 if boundary else (4 if step % 2 == 0 else 3))

            # tail: fc5 + final add + store per ic, 3-deep on PSUM tags.
            fill(10 ** 9)
            for ic in range(IC):
                if ic not in tps:
                    tps[ic] = big_ps(tags[ic % 3], f"yt{ic}")
                    outproj_tail_mm(ic, tps[ic], 4)
                if ic >= 2:
                    j = ic - 2
                    outproj_tail_mm(j, tps[j], 5)
                    outproj_finish(j, tps.pop(j))
            for ic in (IC - 2, IC - 1):
                outproj_tail_mm(ic, tps[ic], 5)
                outproj_finish(ic, tps.pop(ic))

    nc.compile()
    return nc


# host-side W-column permutation within each 128-feature chunk:
# [A feats 0-31 | B feats 0-31 | A feats 32-63 | B feats 32-63]
_PERM128 = np.concatenate([np.arange(0, 32), np.arange(64, 96),
                           np.arange(32, 64), np.arange(96, 128)])
_QK_PERM = np.concatenate([jc * 128 + _PERM128 for jc in range(2 * DC)])


def _prepare(x, W_qkv, b_qkv, W_out, b_out):
    """Build the compiled graph and per-core input maps."""
    bfd = ml_dtypes.bfloat16
    f8 = ml_dtypes.float8_e4m3
    xT = np.ascontiguousarray(np.transpose(x, (0, 2, 1)))                # [B, D, N] f32
    wqkvT = np.ascontiguousarray(W_qkv.T)                                # [D, 3D]
    # fp8 main + residual of x, DR-packed [128, NI, 2, N]
    x8a = xT.astype(f8)
    x8r = (xT - x8a.astype(np.float32)).astype(f8)

    def pack_x(x8):
        return np.ascontiguousarray(
            x8.reshape(B, NI, 2, 128, N).transpose(0, 3, 1, 2, 4)
            .reshape(B, 128, NI * 2 * N))

    xa = pack_x(x8a)
    xr = pack_x(x8r)
    w8 = wqkvT[:, :2 * D][:, _QK_PERM].astype(f8)                        # [D, 2D] permuted
    wqf = np.ascontiguousarray(
        w8.reshape(NI, 2, 128, 2 * D).transpose(2, 0, 1, 3)
        .reshape(128, NI * 4 * D))
    xT16 = np.ascontiguousarray(
        xT.astype(bfd).reshape(B, DC, 128, N).transpose(0, 2, 1, 3)
        .reshape(B, 128, DC * N))
    wv16 = np.ascontiguousarray(
        wqkvT[:, 2 * D:].astype(bfd).reshape(DC, 128, D).transpose(1, 0, 2)
        .reshape(128, DC * D))
    wo16 = np.ascontiguousarray(
        W_out.T.astype(bfd).reshape(DC, 128, D).transpose(1, 0, 2)
        .reshape(128, DC * D))
    has_bqkv = bool(np.any(b_qkv != 0))

    nc = _build(has_bqkv)

    in_maps = []
    for c in range(NCORES):
        m = {"xa": xa[c], "xr": xr[c], "wqf": wqf, "xT": xT16[c],
             "wv": wv16, "wo": wo16}
        if has_bqkv:
            m["bqkv"] = np.ascontiguousarray(
                b_qkv[:2 * D][_QK_PERM]).astype(np.float32)
            m["bv16"] = np.ascontiguousarray(b_qkv[2 * D:]).astype(bfd)
        in_maps.append(m)
    return nc, in_maps


def kernel(x, W_qkv, b_qkv, W_out, b_out):
    from concourse.bass_utils import run_bass_kernel_spmd

    nc, in_maps = _prepare(x, W_qkv, b_qkv, W_out, b_out)

    res = None
    for attempt in range(3):
        try:
            res = run_bass_kernel_spmd(nc, in_maps, core_ids=list(range(NCORES)))
            break
        except Exception:
            if attempt == 2:
                raise
    out = np.stack([res.results[c]["out"] for c in range(NCORES)], axis=0)
    if np.any(b_out != 0):
        out = out + b_out.astype(np.float32)
    return out
